# revision 49
# baseline (speedup 1.0000x reference)
"""BoxPool (NMS-style per-class argmax pooling) Trainium2 Bass kernel, v4.

B=8 batches sharded 1:1 onto 8 NeuronCores (pure data parallel). Per core:
box [4, N], score [C, N] -> pool_mask [C, N] int32 where
pool_mask[c, j] = 1 iff argmax_i (iou_mask[i, j] * score[c, i]) == j
(iou_mask = pairwise IoU >= 0.7, jax argmax first-index tie-break),
class 0 forced to all-ones.

Pipeline: dense O(N^2/2) IoU band-screen in fp16 (DVE 2x/4x packing) with a
conservative slack; ~750 band candidates exactly re-verified in fp32 per
pair; recompacted to the ~110 true pairs; per-class argmax via score
compares + one-hot indicator matmuls.
  A) prologue: box loaded contiguously once; per-tile columns via PE
     transposes; coordinate broadcast rows via PE ones-matmul into a
     packed [128, 6N] row tensor (x1|x2|y1|y2|iota|score); fp16 TAU*area
     row computed on DVE from rows; periodic fp16 iota row
  B) per j-tile t: ScalarE relu cross-terms (fp32->fp16); DVE
     a=min(t1x,w_p), c=min(t1y,h_p) [TS]; oxy=min(ac,tB) [TT];
     itr=ox*oy; e1=itr-ta16_i; m=(e1-(ta_p-SLACK))>=0 [TS]; enc=m*iota16;
     max8 per 1024-chunk -> enc8 (w_i/h_i mins dropped: conservative)
  C) slot decode -> codes j*4096+i, top-24/partition-row (max8 x3 +
     match_replace x2), PE-transpose fold, sparse_gather -> <=1024 pairs
  R) exact fp32 recheck: idx blocks DMA'd from an interleaved layout, ONE
     merged indirect_copy over the packed rows, inter >= TAU*(a_i+a_j),
     codes rebuilt from gathered iota, stride-16 fold, sparse_gather
  D) decode pairs (layout moves via small DMAs), E) per-class score
     compare via one merged indirect_copy (exact tie-break),
  F) suppression scatter via indicator matmul, out = (supp == 0)

GpSimd exotic ops (sparse_gather/indirect_copy) each pay a ~7us invisible
IRAM library load, so they are merged/minimized: exactly 4 exotic calls
(sg1, pack-gather, sg2, GI-gather) plus a pre-warmed sparse lib.
"""

import numpy as np

N = 2134
C = 81
B = 8
NT = (N + 127) // 128  # 17 j-tiles
NLAST = N - 128 * (NT - 1)  # 86
TAU = float(np.float32(0.7) / np.float32(1.7))
SLACK = 10.0
CH = 1024  # i-chunk size for max8 extraction (max chunk degree 7 incl self)
CHUNKS = [(N - 128 * t + CH - 1) // CH for t in range(NT)]
GSTART = [sum(CHUNKS[:t]) for t in range(NT)]
NG = sum(CHUNKS)
NSL = NG * 8
KTOP = 24  # per-partition-row code capacity (measured max 19)
CAP1 = 1024  # band-candidate capacity (measured max ~750)
MAXCODE = float((N - 1) * 4096 + (N - 1))
PCAP = 128  # true-pair capacity (measured ~115)
PW = PCAP // 16
JCH = 5


def build_nc(debug=False):
    import concourse.bacc as bacc
    import concourse.mybir as mybir
    from concourse.tile import TileContext
    import concourse.bass as bass

    fp32 = mybir.dt.float32
    fp16 = mybir.dt.float16
    bf16 = mybir.dt.bfloat16
    i32 = mybir.dt.int32
    i16 = mybir.dt.int16
    u16 = mybir.dt.uint16
    u32 = mybir.dt.uint32
    Alu = mybir.AluOpType
    Act = mybir.ActivationFunctionType

    nc = bacc.Bacc(None, target_bir_lowering=False)

    box = nc.dram_tensor("box", [4, N], fp32, kind="ExternalInput")
    score = nc.dram_tensor("score", [C, N], fp32, kind="ExternalInput")
    out = nc.dram_tensor("out", [C, N], i32, kind="ExternalOutput")
    if debug:
        enc8_dbg = nc.dram_tensor("enc8_dbg", [128, NSL], fp32, kind="ExternalOutput")
        sg1_dbg = nc.dram_tensor("sg1_dbg", [16, CAP1 // 16], fp32, kind="ExternalOutput")
        nf1_dbg = nc.dram_tensor("nf1_dbg", [1, 1], u32, kind="ExternalOutput")
        keep_dbg = nc.dram_tensor("keep_dbg", [128, 128], fp32, kind="ExternalOutput")
        sg2_dbg = nc.dram_tensor("sg2_dbg", [16, PW], fp32, kind="ExternalOutput")
        nf2_dbg = nc.dram_tensor("nf2_dbg", [1, 1], u32, kind="ExternalOutput")

    with TileContext(nc) as tc:
        with (
            tc.tile_pool(name="persist", bufs=1) as pp,
            tc.tile_pool(name="acts", bufs=2) as pa,
            tc.tile_pool(name="mids", bufs=2) as pm,
            tc.tile_pool(name="small", bufs=1) as ps,
            tc.tile_pool(name="psum_t", bufs=2, space="PSUM") as ppt,
            tc.tile_pool(name="psum_acc", bufs=1, space="PSUM") as ppa,
        ):
            # ---------------- stage A: prologue ----------------
            # box loaded contiguously (4 descriptors, fast)
            box_sb = pa.tile([4, N], fp32, tag="tA", name="box_sb")
            nc.sync.dma_start(box_sb[:, :], bass.AP(box, 0, [[N, 4], [1, N]]))


            # PE transpose identity (needed immediately)
            identf = pp.tile([128, 128], fp32, tag="identf")
            onesf = pp.tile([128, 128], fp32, tag="onesf")
            nc.vector.memset(onesf[:, :], 1.0)
            nc.gpsimd.affine_select(
                identf[:, :], onesf[:, :], pattern=[[-1, 128]], compare_op=Alu.is_equal,
                fill=0.0, base=0, channel_multiplier=1,
            )
            onesrow = pp.tile([1, 128], fp32, tag="onesrow")
            nc.vector.memset(onesrow[:, :], 1.0)
            ident4 = pp.tile([4, 4], fp32, tag="ident4")
            ones4 = pp.tile([4, 4], fp32, tag="ones4")
            nc.vector.memset(ones4[:, :], 1.0)
            nc.gpsimd.affine_select(
                ident4[:, :], ones4[:, :], pattern=[[-1, 4]], compare_op=Alu.is_equal,
                fill=0.0, base=0, channel_multiplier=1,
            )

            # per-tile coordinate columns via 17 PE transposes
            colr = pp.tile([128, 4 * NT], fp32, tag="colr")
            nc.vector.memset(colr[:, :], 0.0)
            _ca = colr[:, :]
            for t in range(NT):
                w = min(128, N - 128 * t)
                ptt = ppt.tile([128, 4], fp32, tag="pt", name=f"ptcol{t}")
                nc.tensor.transpose(ptt[0:w, :], box_sb[:, 128 * t : 128 * t + w], ident4[:, :])
                # scatter the 4 columns into colr (strided dst AP)
                nc.scalar.copy(
                    bass.AP(_ca.tensor, _ca.offset + t, [[4 * NT, w], [NT, 4]]),
                    ptt[0:w, :],
                )
            x1c, y1c, x2c, y2c = (colr[:, k * NT : (k + 1) * NT] for k in range(4))

            # packed row tensor [128, 6N]: x1 | x2 | y1 | y2 | iota | score
            rows6 = pp.tile([128, 6 * N], fp32, tag="rows6")
            xr1 = rows6[:, 0:N]
            xr2 = rows6[:, N : 2 * N]
            yr1 = rows6[:, 2 * N : 3 * N]
            yr2 = rows6[:, 3 * N : 4 * N]
            iotar = rows6[:, 4 * N : 5 * N]
            s_sb = rows6[:, 5 * N : 6 * N]
            # coordinate broadcasts via DMA (dram src, stride-0 partitions),
            # split halves across the 3 queues; x rows first (tile-0 ACTs)
            src_of = {0: 0, 1: 2, 2: 1, 3: 3}  # dst block -> box row (x1,x2,y1,y2)
            H = N // 2
            H2 = N - H
            for qi, (r, h0) in enumerate([(1, 0), (0, 0), (1, 1), (0, 1),
                                          (3, 0), (2, 0), (3, 1), (2, 1)]):
                eng = (nc.sync, nc.scalar, nc.gpsimd)[qi % 3]
                o = h0 * H
                w = H2 if h0 else H
                eng.dma_start(
                    rows6[:, r * N + o : r * N + o + w],
                    bass.AP(box, src_of[r] * N + o, [[0, 128], [1, w]]),
                )
            nc.gpsimd.iota(iotar, pattern=[[1, N]], base=1, channel_multiplier=0,
                           allow_small_or_imprecise_dtypes=True)
            nc.scalar.dma_start(rows6[0:C, 5 * N : 6 * N], score[:, :])

            # periodic fp16 iota row: value (k % CH) + 1 (exact in fp16)
            iota16 = pp.tile([128, N], fp16, tag="iota16")
            nc.gpsimd.iota(iota16[:, 0 : 2 * CH], pattern=[[0, 2], [1, CH]], base=1,
                           channel_multiplier=0, allow_small_or_imprecise_dtypes=True)
            nc.gpsimd.iota(iota16[:, 2 * CH : N], pattern=[[1, N - 2 * CH]], base=1,
                           channel_multiplier=0, allow_small_or_imprecise_dtypes=True)

            # column helper scalars
            negx1 = pp.tile([128, NT], fp32, tag="negx1")
            negy1 = pp.tile([128, NT], fp32, tag="negy1")
            wcol = pp.tile([128, NT], fp32, tag="wcol")
            hcol = pp.tile([128, NT], fp32, tag="hcol")
            tacol = pp.tile([128, NT], fp32, tag="tacol")
            tacol_s = pp.tile([128, NT], fp32, tag="tacol_s")
            nc.vector.tensor_scalar_mul(negx1[:, :], x1c, -1.0)
            nc.vector.tensor_scalar_mul(negy1[:, :], y1c, -1.0)
            nc.vector.tensor_sub(wcol[:, :], x2c, x1c)
            nc.vector.tensor_sub(hcol[:, :], y2c, y1c)
            nc.vector.tensor_mul(tacol[:, :], wcol[:, :], hcol[:, :])
            nc.vector.tensor_scalar_mul(tacol[:, :], tacol[:, :], TAU)
            nc.vector.tensor_scalar_sub(tacol_s[:, :], tacol[:, :], SLACK)
            # pad partitions of the last tile (j >= N) must never fire the band
            padind = ps.tile([128, 1], fp32, tag="padind")
            ones1 = ps.tile([128, 1], fp32, tag="ones1")
            nc.vector.memset(ones1[:, :], 1.0)
            nc.gpsimd.affine_select(
                padind[:, :], ones1[:, :], pattern=[[0, 1]], compare_op=Alu.is_ge,
                fill=0.0, base=NLAST - 1, channel_multiplier=-1,
            )
            pc = tacol_s[:, NT - 1 : NT]
            nc.vector.tensor_mul(pc, pc, padind[:, :])
            bigc = ps.tile([128, 1], fp32, tag="bigc")
            nc.vector.tensor_scalar(bigc[:, :], padind[:, :], -1.0, 1.0, Alu.mult, Alu.add)
            nc.vector.tensor_scalar_mul(bigc[:, :], bigc[:, :], 1.0e9)
            nc.vector.tensor_tensor(pc, pc, bigc[:, :], Alu.add)

            # fp16 TAU*area row from the broadcast rows (no DRAM bounce)
            tmpw = pm.tile([128, N], fp16, tag="ac", name="tmpw")
            tmph = pm.tile([128, N], fp16, tag="oxy", name="tmph")
            tar16 = pp.tile([128, N], fp16, tag="tar16")
            nc.vector.tensor_sub(tmpw[:, :], xr2, xr1)
            nc.vector.tensor_sub(tmph[:, :], yr2, yr1)
            nc.vector.scalar_tensor_tensor(tar16[:, :], tmpw[:, :], TAU, tmph[:, :], Alu.mult, Alu.mult)

            # ---------------- stage B: fp16 band screen ----------------
            enc8 = pp.tile([128, NSL], fp16, tag="enc8")
            for t in range(NT):
                i0 = 128 * t
                F = N - i0
                tA = pa.tile([128, 2 * F], fp16, tag="tA", name=f"tA{t}")
                tB = pa.tile([128, 2 * F], fp16, tag="tB", name=f"tB{t}")
                nc.scalar.activation(tA[:, 0:F], rows6[:, N + i0 : 2 * N], Act.Relu, bias=negx1[:, t : t + 1], scale=1.0)
                nc.scalar.activation(tB[:, 0:F], rows6[:, i0:N], Act.Relu, bias=colr[:, 2 * NT + t : 2 * NT + t + 1], scale=-1.0)
                nc.scalar.activation(tA[:, F : 2 * F], rows6[:, 3 * N + i0 : 4 * N], Act.Relu, bias=negy1[:, t : t + 1], scale=1.0)
                nc.scalar.activation(tB[:, F : 2 * F], rows6[:, 2 * N + i0 : 3 * N], Act.Relu, bias=colr[:, 3 * NT + t : 3 * NT + t + 1], scale=-1.0)

                ac = pm.tile([128, 2 * F], fp16, tag="ac", name=f"ac{t}")
                oxy = pm.tile([128, 2 * F], fp16, tag="oxy", name=f"oxy{t}")
                nc.vector.tensor_scalar(ac[:, 0:F], tA[:, 0:F], wcol[:, t : t + 1], None, Alu.min)
                nc.vector.tensor_scalar(ac[:, F : 2 * F], tA[:, F : 2 * F], hcol[:, t : t + 1], None, Alu.min)
                nc.vector.tensor_tensor(oxy[:, :], ac[:, :], tB[:, :], Alu.min)
                nc.vector.tensor_mul(oxy[:, 0:F], oxy[:, 0:F], oxy[:, F : 2 * F])
                nc.vector.tensor_tensor(oxy[:, 0:F], oxy[:, 0:F], tar16[:, i0:N], Alu.subtract)
                m_t = pm.tile([128, F], fp16, tag="m_t", name=f"m{t}")
                nc.vector.tensor_scalar(m_t[:, :], oxy[:, 0:F], tacol_s[:, t : t + 1], 0.0, Alu.subtract, Alu.is_ge)
                nc.vector.tensor_mul(m_t[:, :], m_t[:, :], iota16[:, 0:F])
                for cix in range(CHUNKS[t]):
                    w = min(CH, F - CH * cix)
                    g = GSTART[t] + cix
                    nc.vector.max(enc8[:, 8 * g : 8 * g + 8], m_t[:, CH * cix : CH * cix + w])

            if debug:
                enc8f_d = ps.tile([128, NSL], fp32, tag="enc8f_d")
                nc.vector.tensor_copy(enc8f_d[:, :], enc8[:, :])
                nc.sync.dma_start(enc8_dbg[:, :], enc8f_d[:, :])

            # interleaved coord tensor for the d=4 recheck gather:
            # coordsI[p, 4i+c] = (x1,x2,y1,y2)[c] of box i
            coordsI = pp.tile([128, 4 * N], fp32, tag="coordsI")
            _ci = coordsI[:, :]
            for c4 in range(4):
                nc.scalar.activation(
                    bass.AP(_ci.tensor, _ci.offset + c4, [[4 * N, 128], [4, N]]),
                    rows6[:, c4 * N : (c4 + 1) * N], Act.Identity, bias=0.0, scale=1.0)

            # -------- statics issued after dense (run during it) ----------
            offrowf = ps.tile([128, NSL], fp32, tag="offrowf")
            jmatf = ps.tile([128, NSL], fp32, tag="jmatf")
            for t in range(NT):
                g0 = 8 * GSTART[t]
                g1 = 8 * (GSTART[t] + CHUNKS[t])
                nc.gpsimd.iota(offrowf[:, g0:g1], pattern=[[CH, CHUNKS[t]], [0, 8]],
                               base=128 * t, channel_multiplier=0,
                               allow_small_or_imprecise_dtypes=True)
                nc.gpsimd.iota(jmatf[:, g0:g1], pattern=[[0, g1 - g0]],
                               base=128 * t, channel_multiplier=1,
                               allow_small_or_imprecise_dtypes=True)
            jm4096p1 = ps.tile([128, NSL], fp32, tag="jm4096p1")
            nc.vector.tensor_scalar(jm4096p1[:, :], jmatf[:, :], 4096.0, 1.0, Alu.mult, Alu.add)

            ident = pp.tile([128, 128], bf16, tag="ident")
            ones = pp.tile([128, 128], bf16, tag="ones")
            nc.vector.memset(ones[:, :], 1.0)
            nc.gpsimd.affine_select(
                ident[:, :], ones[:, :], pattern=[[-1, 128]], compare_op=Alu.is_equal,
                fill=0.0, base=0, channel_multiplier=1,
            )
            # pair-index grid [128, 128]: n + 128*(p//16) for recheck validity
            pgi = ps.tile([128, 1], i32, tag="pgi")
            nc.gpsimd.iota(pgi[:, :], pattern=[[1, 1]], base=0, channel_multiplier=1)
            gg = ps.tile([128, 1], i32, tag="gg")
            nc.vector.tensor_scalar(gg[:, :], pgi[:, :], 4, None, Alu.logical_shift_right)
            ggf = ps.tile([128, 1], fp32, tag="ggf")
            nc.vector.tensor_copy(ggf[:, :], gg[:, :])
            pnf = ps.tile([128, 128], fp32, tag="pnf")
            nc.gpsimd.iota(pnf[:, :], pattern=[[1, 128]], base=0, channel_multiplier=0,
                           allow_small_or_imprecise_dtypes=True)
            kbf = ps.tile([128, 1], fp32, tag="kbf")
            nc.vector.tensor_scalar(kbf[:, :], ggf[:, :], 128.0, None, Alu.mult)
            nc.vector.tensor_scalar(pnf[:, :], pnf[:, :], kbf[:, :], None, Alu.add)
            kidx = ps.tile([16, PW], i32, tag="kidx")
            nc.gpsimd.iota(kidx[:, :], pattern=[[16, PW]], base=0, channel_multiplier=1)
            kidxf = ps.tile([16, PW], fp32, tag="kidxf")
            nc.vector.tensor_copy(kidxf[:, :], kidx[:, :])
            zeros16 = ps.tile([16, PW], fp32, tag="zeros16")
            nc.vector.memset(zeros16[:, :], 0.0)
            # warm the sparse_gather library during the dense phase (input has
            # one positive value so num_found > 0)
            wjunk = ps.tile([16, 16], fp32, tag="wjunk")
            nc.gpsimd.memset(wjunk[:, :], -1.0)
            nc.vector.memset(wjunk[0:1, 0:1], 3.0)
            sjunk = ps.tile([16, 8], fp32, tag="sjunk")
            njunk = ps.tile([1, 1], u32, tag="njunk")
            nc.gpsimd.sparse_gather(sjunk[:, :], wjunk[:, :], num_found=njunk[:, :])

            # ---------------- stage C: codes + top-24 + compaction ----------------
            ig = ps.tile([128, NSL], fp32, tag="ig")
            nc.vector.tensor_scalar(ig[:, :], enc8[:, :], 1.0, None, Alu.subtract)
            nc.vector.tensor_tensor(ig[:, :], ig[:, :], offrowf[:, :], Alu.add)
            valid = ps.tile([128, NSL], fp32, tag="validc")
            nc.vector.tensor_scalar(valid[:, :], enc8[:, :], 0.5, None, Alu.is_ge)
            nself = ps.tile([128, NSL], fp32, tag="nself")
            nc.vector.tensor_tensor(nself[:, :], ig[:, :], jmatf[:, :], Alu.not_equal)
            nc.vector.tensor_mul(valid[:, :], valid[:, :], nself[:, :])
            code = ps.tile([128, NSL], fp32, tag="code")
            nc.vector.tensor_tensor(code[:, :], jm4096p1[:, :], ig[:, :], Alu.add)
            nc.vector.tensor_mul(code[:, :], code[:, :], valid[:, :])
            nc.vector.tensor_scalar_sub(code[:, :], code[:, :], 1.0)

            code24 = ps.tile([128, KTOP], fp32, tag="code24")
            nc.vector.max(code24[:, 0:8], code[:, :])
            rep1 = ps.tile([128, NSL], fp32, tag="ig", name="rep1")
            nc.vector.match_replace(rep1[:, :], code24[:, 0:8], code[:, :], -1.0)
            nc.vector.max(code24[:, 8:16], rep1[:, :])
            rep2 = ps.tile([128, NSL], fp32, tag="code", name="rep2")
            nc.vector.match_replace(rep2[:, :], code24[:, 8:16], rep1[:, :], -1.0)
            nc.vector.max(code24[:, 16:24], rep2[:, :])

            ptc1a = ppt.tile([16, 128], fp32, tag="pt", name="ptc1a")
            nc.tensor.transpose(ptc1a[:, :], code24[:, 0:16], identf[:, :])
            ptc1b = ppt.tile([8, 128], fp32, tag="pt", name="ptc1b")
            nc.tensor.transpose(ptc1b[:, :], code24[:, 16:24], identf[:, :])
            bB = ps.tile([8, 128], fp32, tag="bB")
            nc.scalar.copy(bB[:, :], ptc1b[:, :])
            wrapped1 = ps.tile([16, 192], fp32, tag="wrapped1")
            nc.scalar.copy(wrapped1[:, 0:128], ptc1a[:, :])
            nc.scalar.copy(wrapped1[0:8, 128:192], bB[:, 0:64])
            nc.sync.dma_start(wrapped1[8:16, 128:192], bB[:, 64:128])
            sg1 = ps.tile([16, CAP1 // 16], fp32, tag="sg1")
            nf1 = ps.tile([1, 1], u32, tag="nf1")
            nc.gpsimd.sparse_gather(sg1[:, :], wrapped1[:, :], num_found=nf1[:, :])
            if debug:
                nc.sync.dma_start(sg1_dbg[:, :], sg1[:, :])
                nc.sync.dma_start(nf1_dbg[:, :], nf1[:, :])

            # ---------------- stage R: exact fp32 recheck ----------------
            W1 = CAP1 // 16  # 64
            nff1 = ps.tile([1, 1], fp32, tag="nff1")
            nc.vector.tensor_copy(nff1[:, :], nf1[:, :])
            pnb = ppt.tile([128, 1], fp32, tag="pt", name="pnb")
            nc.tensor.matmul(pnb[:, :], onesrow[:, :], nff1[:, :], start=True, stop=True)
            nfb128 = ps.tile([128, 1], fp32, tag="nfb128")
            nc.scalar.copy(nfb128[:, :], pnb[:, :])
            # sanitize codes (garbage tail) then decode i/j
            c1s = ps.tile([16, W1], fp32, tag="c1s")
            nc.vector.tensor_scalar(c1s[:, :], sg1[:, :], 0.0, MAXCODE, Alu.max, Alu.min)
            ci1 = ps.tile([16, W1], i32, tag="ci1")
            nc.vector.tensor_copy(ci1[:, :], c1s[:, :])
            ii1 = ps.tile([16, W1], i32, tag="ii1")
            jj1 = ps.tile([16, W1], i32, tag="jj1")
            nc.vector.tensor_scalar(ii1[:, :], ci1[:, :], 4095, None, Alu.bitwise_and)
            nc.vector.tensor_scalar(jj1[:, :], ci1[:, :], 12, None, Alu.logical_shift_right)
            # interleaved u16 idx layout: col 16a+b = ii[f=8a+b], 16a+8+b = jj[f]
            iijj16 = ps.tile([16, 128], i16, tag="iijj16")
            _z = iijj16[:, :]
            nc.vector.tensor_copy(
                bass.AP(_z.tensor, _z.offset, [[128, 16], [16, 8], [1, 8]]), ii1[:, :])
            nc.vector.tensor_copy(
                bass.AP(_z.tensor, _z.offset + 8, [[128, 16], [16, 8], [1, 8]]), jj1[:, :])
            idxIJ = ps.tile([128, 16], i16, tag="idxIJ")
            for k in range(8):
                eng = (nc.sync, nc.scalar, nc.gpsimd)[k % 3]
                eng.dma_start(idxIJ[16 * k : 16 * (k + 1), :], iijj16[:, 16 * k : 16 * k + 16])
            # one d=4 gather: out col 4n+c = coord-c of pair-side n (i:0-127, j:128-255)
            cg = ps.tile([128, 1024], fp32, tag="cg")
            nc.gpsimd.ap_gather(cg[:, :], coordsI[:, :], idxIJ[:, :],
                                channels=128, num_elems=N, d=4, num_idxs=256)
            ioout = ps.tile([128, 256], fp32, tag="ioout")
            nc.gpsimd.ap_gather(ioout[:, :], rows6[:, 4 * N : 5 * N], idxIJ[:, :],
                                channels=128, num_elems=N, d=1, num_idxs=256)
            # exact recheck; cg col 4n+c -> strided column views
            _cg = cg[:, :]
            def cgap(c, n0, cnt):
                return bass.AP(_cg.tensor, _cg.offset + 4 * n0 + c, [[1024, 128], [4, cnt]])
            wh = ps.tile([128, 256], fp32, tag="wh")
            nc.vector.tensor_tensor(wh[:, :], cgap(1, 0, 256), cgap(0, 0, 256), Alu.subtract)
            hh = ps.tile([128, 256], fp32, tag="hh")
            nc.vector.tensor_tensor(hh[:, :], cgap(3, 0, 256), cgap(2, 0, 256), Alu.subtract)
            taa = ps.tile([128, 256], fp32, tag="taa")
            nc.vector.scalar_tensor_tensor(taa[:, :], wh[:, :], TAU, hh[:, :], Alu.mult, Alu.mult)
            mnx = ps.tile([128, 128], fp32, tag="mnx")
            mxx = ps.tile([128, 128], fp32, tag="mxx")
            nc.vector.tensor_tensor(mnx[:, :], cgap(1, 0, 128), cgap(1, 128, 128), Alu.min)
            nc.vector.tensor_tensor(mxx[:, :], cgap(0, 0, 128), cgap(0, 128, 128), Alu.max)
            oxe = ps.tile([128, 128], fp32, tag="oxe")
            nc.vector.tensor_sub(oxe[:, :], mnx[:, :], mxx[:, :])
            mny = ps.tile([128, 128], fp32, tag="mnx", name="mny")
            mxy = ps.tile([128, 128], fp32, tag="mxx", name="mxy")
            nc.vector.tensor_tensor(mny[:, :], cgap(3, 0, 128), cgap(3, 128, 128), Alu.min)
            nc.vector.tensor_tensor(mxy[:, :], cgap(2, 0, 128), cgap(2, 128, 128), Alu.max)
            oye = ps.tile([128, 128], fp32, tag="oye")
            nc.vector.tensor_sub(oye[:, :], mny[:, :], mxy[:, :])
            itre = ps.tile([128, 128], fp32, tag="mnx", name="itre")
            nc.vector.scalar_tensor_tensor(itre[:, :], oxe[:, :], 0.0, oye[:, :], Alu.max, Alu.mult)
            d1e = ps.tile([128, 128], fp32, tag="mxx", name="d1e")
            nc.vector.tensor_sub(d1e[:, :], itre[:, :], taa[:, 0:128])
            keep = ps.tile([128, 128], fp32, tag="keep")
            nc.vector.tensor_tensor(keep[:, :], d1e[:, :], taa[:, 128:256], Alu.is_ge)
            validp = ps.tile([128, 128], fp32, tag="oxe", name="validp")
            nc.vector.tensor_scalar(validp[:, :], pnf[:, :], nfb128[:, :], None, Alu.is_lt)
            nc.vector.tensor_mul(keep[:, :], keep[:, :], validp[:, :])
            if debug:
                nc.sync.dma_start(keep_dbg[:, :], keep[:, :])

            # rebuild codes from gathered iota (i+1): (j*4096 + i + 1)*keep - 1
            jv = ps.tile([128, 128], fp32, tag="mnx", name="jv")
            nc.vector.tensor_scalar(jv[:, :], ioout[:, 128:256], 1.0, 4096.0, Alu.subtract, Alu.mult)
            codes2 = ps.tile([128, 128], fp32, tag="codes2")
            nc.vector.tensor_tensor(codes2[:, :], jv[:, :], ioout[:, 0:128], Alu.add)
            nc.vector.tensor_mul(codes2[:, :], codes2[:, :], keep[:, :])
            nc.vector.tensor_scalar_sub(codes2[:, :], codes2[:, :], 1.0)

            # fold: rows {0,16,...,112} hold each core's 128 pair codes
            wrapped2 = ps.tile([16, 128], fp32, tag="wrapped2")
            nc.vector.memset(wrapped2[:, :], -1.0)
            _c2 = codes2[:, :]
            nc.sync.dma_start(
                wrapped2[0:8, :],
                bass.AP(_c2.tensor, _c2.offset, [[16 * 128, 8], [1, 128]]),
            )
            sgout = ps.tile([16, PW], fp32, tag="sgout")
            nf = ps.tile([1, 1], u32, tag="nf")
            nc.vector.memset(sgout[:, :], -1.0)
            nc.gpsimd.sparse_gather(sgout[:, :], wrapped2[:, :], num_found=nf[:, :])
            if debug:
                nc.sync.dma_start(sg2_dbg[:, :], sgout[:, :])
                nc.sync.dma_start(nf2_dbg[:, :], nf[:, :])

            # ---------------- stage D: decode pairs ----------------
            nff = ps.tile([1, 1], fp32, tag="nff")
            nc.vector.tensor_copy(nff[:, :], nf[:, :])
            pnb2 = ppt.tile([128, 1], fp32, tag="pt", name="pnb2")
            nc.tensor.matmul(pnb2[:, :], onesrow[:, :], nff[:, :], start=True, stop=True)
            nfb = ps.tile([16, 1], fp32, tag="nfb")
            nc.scalar.copy(nfb[:, :], pnb2[0:16, :])
            valid2 = ps.tile([16, PW], i32, tag="valid2")
            nc.vector.tensor_scalar(valid2[:, :], kidxf[:, :], nfb[:, :], None, Alu.is_lt)
            codes = ps.tile([16, PW], fp32, tag="codes")
            nc.vector.select(codes[:, :], valid2[:, :], sgout[:, :], zeros16[:, :])
            nc.vector.tensor_scalar_max(codes[:, :], codes[:, :], 0.0)

            ci = ps.tile([16, PW], i32, tag="ci")
            jj_i = ps.tile([16, PW], i32, tag="jj_i")
            ii_i = ps.tile([16, PW], i32, tag="ii_i")
            nc.vector.tensor_copy(ci[:, :], codes[:, :])
            nc.vector.tensor_scalar(jj_i[:, :], ci[:, :], 12, None, Alu.logical_shift_right)
            nc.vector.tensor_scalar(ii_i[:, :], ci[:, :], 4095, None, Alu.bitwise_and)
            # idx for the merged GI gather: io-block then score-block
            iju = ps.tile([16, 2 * PW], i16, tag="iju")
            nc.vector.tensor_copy(iju[:, 0:PW], ii_i[:, :])
            nc.vector.tensor_copy(iju[:, PW : 2 * PW], jj_i[:, :])
            ijGI = ps.tile([128, 16], i16, tag="ijGI")
            for k in range(8):
                eng = (nc.sync, nc.scalar, nc.gpsimd)[k % 3]
                eng.dma_start(ijGI[16 * k : 16 * (k + 1), :], iju[:, :])
            # per-partition (i, j) columns for stage F: interleaved + 8 DMAs
            ijpair2 = ps.tile([16, 2 * PW], fp32, tag="ijpair2")
            _p2 = ijpair2[:, :]
            nc.vector.tensor_copy(
                bass.AP(_p2.tensor, _p2.offset, [[2 * PW, 16], [2, PW]]), ii_i[:, :])
            nc.vector.tensor_copy(
                bass.AP(_p2.tensor, _p2.offset + 1, [[2 * PW, 16], [2, PW]]), jj_i[:, :])
            dcol = ps.tile([128, 2], fp32, tag="dcol")
            for k in range(8):
                eng = (nc.scalar, nc.gpsimd, nc.sync)[k % 3]
                eng.dma_start(dcol[16 * k : 16 * (k + 1), :], ijpair2[:, 2 * k : 2 * k + 2])
            # per-pair tie-break flags (i<j / i>j) as columns, broadcast to
            # [128, PCAP] via PE transpose + ones-matmul (replaces iota gather)
            tbf = ps.tile([128, 1], fp32, tag="tbf")
            tbr = ps.tile([128, 1], fp32, tag="tbr")
            nc.vector.tensor_tensor(tbf[:, :], dcol[:, 0:1], dcol[:, 1:2], Alu.is_lt)
            nc.vector.tensor_tensor(tbr[:, :], dcol[:, 0:1], dcol[:, 1:2], Alu.is_gt)
            ptbf = ppt.tile([1, 128], fp32, tag="pt", name="ptbf")
            nc.tensor.transpose(ptbf[:, :], tbf[:, :], identf[:, :])
            tbfT = ps.tile([1, 128], fp32, tag="tbfT")
            nc.scalar.copy(tbfT[:, :], ptbf[:, :])
            ptbr = ppt.tile([1, 128], fp32, tag="pt", name="ptbr")
            nc.tensor.transpose(ptbr[:, :], tbr[:, :], identf[:, :])
            tbrT = ps.tile([1, 128], fp32, tag="tbrT")
            nc.scalar.copy(tbrT[:, :], ptbr[:, :])
            pbf = ppt.tile([128, 128], fp32, tag="pt", name="pbf")
            nc.tensor.matmul(pbf[:, :], onesrow[:, :], tbfT[:, :], start=True, stop=True)
            tbFs = ps.tile([128, 128], fp32, tag="wh", name="tbFs")
            nc.scalar.copy(tbFs[:, :], pbf[:, :])
            pbr = ppt.tile([128, 128], fp32, tag="pt", name="pbr")
            nc.tensor.matmul(pbr[:, :], onesrow[:, :], tbrT[:, :], start=True, stop=True)
            tbRs = ps.tile([128, 128], fp32, tag="hh", name="tbRs")
            nc.scalar.copy(tbRs[:, :], pbr[:, :])

            # indicator rows for stage F (depend only on dcol; issued early to
            # overlap the score gather)
            iipmf = ps.tile([128, 1], fp32, tag="iipmf")
            jjpmf = ps.tile([128, 1], fp32, tag="jjpmf")
            nc.vector.tensor_scalar_add(iipmf[:, :], dcol[:, 0:1], 1.0)
            nc.vector.tensor_scalar_add(jjpmf[:, :], dcol[:, 1:2], 1.0)
            ind_f = pm.tile([128, N], bf16, tag="ind_f", name="ind_f", bufs=1)
            ind_r = pm.tile([128, N], bf16, tag="ind_r", name="ind_r", bufs=1)
            nc.vector.tensor_scalar(ind_f[:, :], iotar, jjpmf[:, 0:1], None, Alu.is_equal)
            nc.vector.tensor_scalar(ind_r[:, :], iotar, iipmf[:, 0:1], None, Alu.is_equal)

            # ---------------- stage E: merged gather + compare ----------------
            gio = ps.tile([128, 256], fp32, tag="cg", name="gio")
            nc.gpsimd.ap_gather(gio[:, :], rows6[:, 5 * N : 6 * N], ijGI[:, :],
                                channels=128, num_elems=N, d=1, num_idxs=256)
            G_i = gio[:, 0:PCAP]
            G_j = gio[:, PCAP : 2 * PCAP]

            eq = ps.tile([128, PCAP], fp32, tag="eq")
            beat_f = ps.tile([128, PCAP], bf16, tag="beat_f")
            beat_r = ps.tile([128, PCAP], bf16, tag="beat_r")
            nc.vector.tensor_tensor(eq[:, :], G_i, G_j, Alu.is_equal)
            gt = ps.tile([128, PCAP], fp32, tag="cmp_t", name="gt")
            e_f = ps.tile([128, PCAP], fp32, tag="cmp_e", name="e_f")
            nc.vector.tensor_tensor(gt[:, :], G_i, G_j, Alu.is_gt)
            nc.vector.tensor_tensor(e_f[:, :], eq[:, :], tbFs[:, :], Alu.mult)
            nc.vector.tensor_tensor(beat_f[:, :], gt[:, :], e_f[:, :], Alu.add)
            lt = ps.tile([128, PCAP], fp32, tag="cmp_t", name="lt")
            e_r = ps.tile([128, PCAP], fp32, tag="cmp_e", name="e_r")
            nc.vector.tensor_tensor(lt[:, :], G_i, G_j, Alu.is_lt)
            nc.vector.tensor_tensor(e_r[:, :], eq[:, :], tbRs[:, :], Alu.mult)
            nc.vector.tensor_tensor(beat_r[:, :], lt[:, :], e_r[:, :], Alu.add)

            beatT_f = ps.tile([128, C], bf16, tag="beatT_f")
            beatT_r = ps.tile([128, C], bf16, tag="beatT_r")
            pt = ppt.tile([128, 128], bf16, tag="pt", name="ptE")
            nc.tensor.transpose(pt[:, :], beat_f[:, :], ident[:, :])
            nc.scalar.copy(beatT_f[:, :], pt[:, 0:C])
            pt2 = ppt.tile([128, 128], bf16, tag="pt", name="ptE2")
            nc.tensor.transpose(pt2[:, :], beat_r[:, :], ident[:, :])
            nc.scalar.copy(beatT_r[:, :], pt2[:, 0:C])

            # ---------------- stage F: indicator matmul scatter ----------------
            psums = [ppa.tile([128, 512], fp32, tag=f"acc{jc}", name=f"acc{jc}") for jc in range(JCH)]
            for jc in range(JCH):
                w = min(512, N - 512 * jc)
                nc.tensor.matmul(
                    psums[jc][0:C, 0:w], beatT_f[:, :],
                    ind_f[:, 512 * jc : 512 * jc + w], start=True, stop=False,
                )
                nc.tensor.matmul(
                    psums[jc][0:C, 0:w], beatT_r[:, :],
                    ind_r[:, 512 * jc : 512 * jc + w], start=False, stop=True,
                )
                osb = pm.tile([128, 512], i32, tag="osb", name=f"osb{jc}", bufs=2)
                nc.vector.tensor_scalar(osb[0:C, 0:w], psums[jc][0:C, 0:w], 0.0, None, Alu.is_equal)
                nc.vector.memset(osb[0:1, 0:w], 1)
                eng = (nc.sync, nc.scalar, nc.gpsimd)[jc % 3]
                eng.dma_start(
                    bass.AP(out, 512 * jc, [[N, C], [1, w]]),
                    osb[0:C, 0:w],
                )

    nc.finalize()
    return nc


_CACHED = {}


def _get_nc(debug=False):
    if debug not in _CACHED:
        _CACHED[debug] = build_nc(debug=debug)
    return _CACHED[debug]


def kernel(box: np.ndarray, score: np.ndarray) -> np.ndarray:
    """Full inputs: box [8,4,2134] f32, score [8,81,2134] f32.
    Returns pool_mask [8,81,2134] int32."""
    from concourse.bass_utils import run_bass_kernel_spmd

    box = np.ascontiguousarray(box, dtype=np.float32)
    score = np.ascontiguousarray(score, dtype=np.float32)
    nc = _get_nc()
    in_maps = [{"box": box[b], "score": score[b]} for b in range(B)]
    res = run_bass_kernel_spmd(nc, in_maps, core_ids=list(range(B)))
    return np.stack([res.results[b]["out"] for b in range(B)], axis=0)
